# revision 1
# baseline (speedup 1.0000x reference)
"""FP4Linear forward for Trainium2, 8-way tensor-parallel.

y = x @ w_t  with x:[8192,4096] f32 and w_t:[4096,16384] f32 (w_t is the
exactly-consistent dequantized transposed weight supplied by the problem, so
no on-chip dequantization is needed).

Sharding (column-parallel per the hint): w_t is split along out_features into
8 shards of 2048; every core holds a replica of x and computes its own
y[:, c*2048:(c+1)*2048]; the host concatenates the slices.

Per-core kernel (float32r matmuls — full fp32 operand bits, PE runs them at
bf16-class rate; accumulation fp32 in PSUM):
  - x is pre-laid-out on host as [64, 128(k), 32(ko), 128(m)] so each m-tile
    load is one fully contiguous 2 MiB DMA.
  - w shard is pre-laid-out as [2, 128(k), 32(ko), 1024(n)]: each n-half is
    contiguous and cached whole in SBUF (128 KiB/partition); the m-loop runs
    twice (once per half) with x streamed again.
  - Inner loop: for each (m-tile, 512-wide n-tile): 32 accumulating matmuls
    over the contraction dim, PSUM -> SBUF copy on the vector engine, store
    DMA on the scalar engine's HWDGE ring (keeps the sync ring free for
    loads).
"""

import numpy as np

import concourse.mybir as mybir
import concourse.tile as tile
from concourse import bacc
from concourse.bass_utils import run_bass_kernel_spmd

P = 128
M_FULL, K_FULL, N_FULL = 8192, 4096, 16384
N_CORES = 8
N_PER = N_FULL // N_CORES  # 2048
KO = K_FULL // P  # 32
MT = M_FULL // P  # 64
FD = 512  # matmul moving free dim == one PSUM bank of fp32
HALVES = 2
NH = N_PER // HALVES  # 1024
KC = 8  # ko-chunk per w DMA (contiguous 32 KiB/partition)

_CACHE = {}


def build_nc(repeat=1):
    nc = bacc.Bacc("TRN2", target_bir_lowering=False, debug=False)
    dt = mybir.dt.float32r
    xd = nc.dram_tensor("x4", [MT, P, KO, P], dt, kind="ExternalInput")
    wd = nc.dram_tensor("w3", [HALVES, P, KO, NH], dt, kind="ExternalInput")
    yd = nc.dram_tensor("y3", [MT, P, N_PER], mybir.dt.float32,
                        kind="ExternalOutput")
    with tile.TileContext(nc) as tc:
        with (
            tc.tile_pool(name="wpool", bufs=1) as wpool,
            tc.tile_pool(name="xpool", bufs=3) as xpool,
            tc.tile_pool(name="opool", bufs=3) as opool,
            tc.tile_pool(name="psum", bufs=8, space="PSUM") as psum,
        ):
            for _rep in range(repeat):
                for h in range(HALVES):
                    wt = wpool.tile([P, KO, NH], dt, tag="wt")
                    for c in range(KO // KC):
                        nc.sync.dma_start(
                            wt[:, c * KC : (c + 1) * KC, :],
                            wd[h, :, c * KC : (c + 1) * KC, :],
                        )
                    for mt in range(MT):
                        xt = xpool.tile([P, KO, P], dt, tag="xt")
                        nc.sync.dma_start(xt[:], xd[mt])
                        ot = opool.tile([P, NH], mybir.dt.float32, tag="ot")
                        for nt in range(NH // FD):
                            ps = psum.tile([P, FD], mybir.dt.float32, tag="ps")
                            for ko in range(KO):
                                nc.tensor.matmul(
                                    ps[:],
                                    xt[:, ko, :],
                                    wt[:, ko, nt * FD : (nt + 1) * FD],
                                    start=(ko == 0),
                                    stop=(ko == KO - 1),
                                )
                            nc.vector.tensor_copy(
                                ot[:, nt * FD : (nt + 1) * FD], ps[:]
                            )
                        nc.scalar.dma_start(
                            yd[mt][:, h * NH : (h + 1) * NH], ot[:]
                        )
    nc.compile()
    return nc


def prep_x(x):
    # [M, K] -> [MT, P(k), KO, P(m)]; elem [mt, p, ko, m] = x[mt*128+m, ko*128+p]
    a = np.ascontiguousarray(x, dtype=np.float32)
    return np.ascontiguousarray(a.reshape(MT, P, KO, P).transpose(0, 3, 2, 1))


def prep_w(w_slice):
    # [K, N_PER] -> [HALVES, P(k), KO, NH]; [h,p,ko,n] = w[ko*128+p, h*NH+n]
    a = np.ascontiguousarray(w_slice, dtype=np.float32)
    return np.ascontiguousarray(
        a.reshape(KO, P, HALVES, NH).transpose(2, 1, 0, 3)
    )


def kernel(x, w_q, w_os, w_is, w_t):
    if "nc" not in _CACHE:
        _CACHE["nc"] = build_nc(1)
    nc = _CACHE["nc"]

    xprep = prep_x(x)
    in_maps = [
        {"x4": xprep, "w3": prep_w(w_t[:, c * N_PER : (c + 1) * N_PER])}
        for c in range(N_CORES)
    ]
    res = run_bass_kernel_spmd(nc, in_maps, core_ids=list(range(N_CORES)))

    y = np.empty((M_FULL, N_FULL), dtype=np.float32)
    for c in range(N_CORES):
        y[:, c * N_PER : (c + 1) * N_PER] = (
            res.results[c]["y3"].reshape(M_FULL, N_PER)
        )
    return y

